# revision 1
# baseline (speedup 1.0000x reference)
"""CheckersGPT dense transformer forward pass on 8 Trainium2 NeuronCores.

Strategy: pure data-parallel over the batch dim (16 batches -> 2 per core).
Each core runs the full 6-layer transformer on its 512 tokens (2 batches x
256 tokens) with all weights replicated. No collectives needed; the final
[2, 512] probability slices are concatenated on the host.

Numerics: matmul operands are bf16 (weights pre-converted on host; activation
operands rounded to bf16 on the PSUM->SBUF evacuation), accumulation is fp32
in PSUM, and all softmax / layernorm / residual math is fp32. Set
CKGPT_MM_DT=f32r or f32 for full-width fallbacks.

Layout convention per core (P=128 partitions):
  xT   [128, 4, 512]  : x transposed; chunk c holds embed dims [128c,128c+128),
                        free dim = 512 tokens. Used as matmul lhsT/rhs.
  xN   [128, 4, 512]  : x natural; chunk c holds tokens [128c,128c+128),
                        free dim = 512 embed. Used for residuals / LN (fp32).
All matmuls are out = lhsT.T @ rhs with contraction on the partition dim.
The last layer only computes Q/attention/FFN for the final token of each
batch (the head reads only x[:, -1, :]).
"""

import os
import numpy as np
from contextlib import ExitStack

import ml_dtypes
import concourse.bass as bass
import concourse.tile as tile
from concourse import bacc, mybir
from concourse.bass_utils import run_bass_kernel_spmd

F32 = mybir.dt.float32
BF16 = mybir.dt.bfloat16
I32 = mybir.dt.int32
AX = mybir.AxisListType
ALU = mybir.AluOpType
ACTF = mybir.ActivationFunctionType

V, E, L, H, B, T = 512, 512, 6, 8, 16, 256
NCORES = 8
BPC = B // NCORES          # batches per core
TOK = BPC * T              # tokens per core
P = 128
EC = E // P                # embed chunks of 128
TC = TOK // P              # token chunks of 128
NEG = -1e9
EPS = 1e-5

MODE = os.environ.get("CKGPT_MM_DT", "bf16")   # bf16 | f32r | f32
# dtype of matmul-operand SBUF tiles and of the weights in DRAM
MM_DT = {"bf16": BF16, "f32r": F32, "f32": F32}[MODE]
MM_CAST = mybir.dt.float32r if MODE == "f32r" else None
NP_WDT = ml_dtypes.bfloat16 if MODE == "bf16" else np.float32

_CACHE = {}


def _c(ap):
    """Cast an AP for matmul input (f32r mode only)."""
    return ap.bitcast(MM_CAST) if MM_CAST is not None else ap


def _mm(nc, out, lhsT, rhs, start, stop):
    nc.tensor.matmul(out, _c(lhsT), _c(rhs), start=start, stop=stop)


def _build(nlayers=L, reps=1, last_opt=True):
    nc = bacc.Bacc("TRN2", target_bir_lowering=False, debug=False, num_devices=NCORES)

    def din(name, shape, dtype=F32):
        return nc.dram_tensor(name, list(shape), dtype, kind="ExternalInput").ap()

    tok = din("tok", [P, TC], I32)            # token ids, p-major within chunks
    emb = din("emb", [V, E])
    pe2 = din("pe2", [TOK, E])                # positional encoding tiled over BPC
    wq = din("wq", [L, H, E, E], MM_DT)
    wk = din("wk", [L, H, E, E], MM_DT)
    wv = din("wv", [L, H, E, E], MM_DT)
    wo = din("wo", [L, H * E, E], MM_DT)
    bo = din("bo", [L, E])
    ln1w = din("ln1w", [L, E])
    ln1b = din("ln1b", [L, E])
    ln2w = din("ln2w", [L, E])
    ln2b = din("ln2b", [L, E])
    ff1w = din("ff1w", [L, E, E], MM_DT)
    ff1b = din("ff1b", [L, E])
    ff2w = din("ff2w", [L, E, E], MM_DT)
    ff2b = din("ff2b", [L, E])
    wout = din("wout", [E, V], MM_DT)
    bout = din("bout", [V])
    masks = din("masks", [2, P, T])           # additive causal mask per i-chunk
    ident = din("ident", [P, P])
    probs = nc.dram_tensor("probs", [BPC, V], F32, kind="ExternalOutput").ap()
    aps = (emb, pe2, wq, wk, wv, wo, bo, ln1w, ln1b, ln2w, ln2b,
           ff1w, ff1b, ff2w, ff2b, wout, bout, masks, ident, probs, tok)

    with tile.TileContext(nc) as tc, ExitStack() as ctx:
        if reps > 1:
            with tc.For_i(0, reps, 1):
                _emit(nc, tc, ctx, aps, nlayers, last_opt)
        else:
            _emit(nc, tc, ctx, aps, nlayers, last_opt)

    nc.compile()
    return nc


def _emit(nc, tc, ctx, aps, nlayers, last_opt):
    (emb, pe2, wq, wk, wv, wo, bo, ln1w, ln1b, ln2w, ln2b,
     ff1w, ff1b, ff2w, ff2b, wout, bout, masks, ident, probs, tok) = aps
    ep = ctx.enter_context

    const = ep(tc.tile_pool(name="const", bufs=1))
    wqkv_p = ep(tc.tile_pool(name="wqkv", bufs=4))
    wo_p = ep(tc.tile_pool(name="wo", bufs=2))
    wff_p = ep(tc.tile_pool(name="wff", bufs=1))
    bias_p = ep(tc.tile_pool(name="bias", bufs=1))
    act_p = ep(tc.tile_pool(name="act", bufs=2))
    qkv_p = ep(tc.tile_pool(name="qkvact", bufs=5))
    ot_p = ep(tc.tile_pool(name="ot", bufs=3))
    ff_p = ep(tc.tile_pool(name="ffact", bufs=3))
    tmp_p = ep(tc.tile_pool(name="tmp", bufs=3))
    esb_p = ep(tc.tile_pool(name="esb", bufs=6))
    attT_p = ep(tc.tile_pool(name="attT", bufs=3))
    st_p = ep(tc.tile_pool(name="stats", bufs=8))
    out_p = ep(tc.tile_pool(name="outp", bufs=1))

    ppb = ep(tc.tile_pool(name="ppb", bufs=3, space="PSUM"))
    ppa = ep(tc.tile_pool(name="ppa", bufs=3, space="PSUM"))
    ppt = ep(tc.tile_pool(name="ppt", bufs=2, space="PSUM"))

    # ---- constants ----
    ident_t = const.tile([P, P], F32)
    nc.sync.dma_start(out=ident_t[:], in_=ident)
    mask_t = const.tile([P, 2, T], F32)
    nc.sync.dma_start(out=mask_t[:], in_=masks.rearrange("c p j -> p c j"))
    eps_t = const.tile([P, 1], F32)
    nc.vector.memset(eps_t[:], EPS)
    tok_t = const.tile([P, TC], I32)
    nc.sync.dma_start(out=tok_t[:], in_=tok)

    def wtile(pool, dram2d, tag):
        t = pool.tile([P, EC, E], MM_DT, tag=tag)
        nc.sync.dma_start(
            out=_c(t[:]),
            in_=_c(dram2d.rearrange("(c p) o -> p c o", p=P)),
        )
        return t

    def bbcast(vec_ap, tag="bias"):
        t = bias_p.tile([P, E], F32, tag=tag)
        nc.sync.dma_start(out=t[:], in_=vec_ap.partition_broadcast(P))
        return t

    def evac(dst, src, use_act):
        """PSUM -> SBUF copy (dtype conversion happens on write)."""
        if use_act:
            nc.scalar.copy(_c(dst), src)
        else:
            nc.vector.tensor_copy(_c(dst), src)

    def transpose_into(dstT, srcN, nsrc_chunks=TC):
        # dstT[:, bb, a*P:(a+1)*P] = srcN[:, a, bb*P:(bb+1)*P].T
        for a in range(nsrc_chunks):
            for bb in range(EC):
                tp = ppt.tile([P, P], F32, tag="tp")
                nc.tensor.transpose(
                    tp[:], srcN[:, a, bb * P : (bb + 1) * P], ident_t[:]
                )
                evac(dstT[:, bb, a * P : (a + 1) * P], tp[:], (a + bb) % 2)

    def layernorm(src, dst, w_b, b_b, tag, rows=P):
        # dst = (src - mean)/sqrt(var+eps) * w + b ; src/dst [rows, E] fp32
        stt = st_p.tile([P, 6], F32, tag=tag + "s")
        nc.vector.bn_stats(out=stt[:rows], in_=src)
        mv = st_p.tile([P, 2], F32, tag=tag + "m")
        nc.vector.bn_aggr(out=mv[:rows], in_=stt[:rows])
        sd = st_p.tile([P, 1], F32, tag=tag + "d")
        nc.scalar.activation(
            out=sd[:rows], in_=mv[:rows, 1:2], func=ACTF.Sqrt, bias=eps_t[:rows, 0:1]
        )
        rs = st_p.tile([P, 1], F32, tag=tag + "r")
        nc.vector.reciprocal(out=rs[:rows], in_=sd[:rows])
        nc.vector.tensor_scalar(
            out=dst, in0=src, scalar1=mv[:rows, 0:1], scalar2=rs[:rows],
            op0=ALU.subtract, op1=ALU.mult,
        )
        nc.gpsimd.tensor_mul(out=dst, in0=dst, in1=w_b[:rows, :])
        nc.gpsimd.tensor_add(out=dst, in0=dst, in1=b_b[:rows, :])

    # ---- embedding gather + positional encoding ----
    xN = act_p.tile([P, TC, E], F32, tag="xN")
    for c in range(TC):
        nc.gpsimd.indirect_dma_start(
            out=xN[:, c, :], out_offset=None, in_=emb,
            in_offset=bass.IndirectOffsetOnAxis(ap=tok_t[:, c : c + 1], axis=0),
        )
    pe_t = ff_p.tile([P, TC, E], F32, tag="ff32")
    nc.sync.dma_start(out=pe_t[:], in_=pe2.rearrange("(c p) o -> p c o", p=P))
    for c in range(TC):
        nc.vector.tensor_add(out=xN[:, c, :], in0=xN[:, c, :], in1=pe_t[:, c, :])
    xT = act_p.tile([P, EC, TOK], MM_DT, tag="xT")
    transpose_into(xT, xN)

    for l in range(nlayers):
        last = last_opt and (l == L - 1) and (nlayers == L)
        bo_b = bbcast(bo[l], "b_bo")
        ln1w_b = bbcast(ln1w[l], "b_l1w")
        ln1b_b = bbcast(ln1b[l], "b_l1b")
        ln2w_b = bbcast(ln2w[l], "b_l2w")
        ln2b_b = bbcast(ln2b[l], "b_l2b")
        ff2b_b = bbcast(ff2b[l], "b_f2")
        ff1b_t = bias_p.tile([P, EC], F32, tag="b_f1")
        nc.sync.dma_start(out=ff1b_t[:], in_=ff1b[l].rearrange("(c p) -> p c", p=P))

        if not last:
            attn_acc = act_p.tile([P, TC, E], F32, tag="acc")
        else:
            pw_last = ppb.tile([BPC, E], F32, tag="ppb")

        for h in range(H):
            wq_t = wtile(wqkv_p, wq[l, h], "wqkv")
            wk_t = wtile(wqkv_p, wk[l, h], "wqkv")
            wv_t = wtile(wqkv_p, wv[l, h], "wqkv")
            wo_t = wtile(wo_p, wo[l, h * E : (h + 1) * E, :], "wo")

            # K^T: [o-chunk, tokens] ; V natural: [t-chunk, o]
            KT = qkv_p.tile([P, EC, TOK], MM_DT, tag="qkv")
            for oc in range(EC):
                ps = ppb.tile([P, TOK], F32, tag="ppb")
                for ec in range(EC):
                    _mm(nc, ps[:], wk_t[:, ec, oc * P : (oc + 1) * P],
                        xT[:, ec, :], ec == 0, ec == EC - 1)
                evac(KT[:, oc, :], ps[:], True)
            VN = qkv_p.tile([P, TC, E], MM_DT, tag="qkv")
            for tcc in range(TC):
                ps = ppb.tile([P, E], F32, tag="ppb")
                for ec in range(EC):
                    _mm(nc, ps[:], xT[:, ec, tcc * P : (tcc + 1) * P],
                        wv_t[:, ec, :], ec == 0, ec == EC - 1)
                evac(VN[:, tcc, :], ps[:], False)

            if not last:
                QT = qkv_p.tile([P, EC, TOK], MM_DT, tag="qkv")
                for oc in range(EC):
                    ps = ppb.tile([P, TOK], F32, tag="ppb")
                    for ec in range(EC):
                        _mm(nc, ps[:], wq_t[:, ec, oc * P : (oc + 1) * P],
                            xT[:, ec, :], ec == 0, ec == EC - 1)
                    evac(QT[:, oc, :], ps[:], True)

                for b in range(BPC):
                    t0 = b * T
                    # --- energy + softmax, causal-skipped ---
                    # i-chunk 0 attends j in [0,128); i-chunk 1 attends [0,256)
                    pse0 = ppa.tile([P, T], F32, tag="ppa")
                    for ec in range(EC):
                        _mm(nc, pse0[:, 0:P], QT[:, ec, (2 * b) * P : (2 * b + 1) * P],
                            KT[:, ec, t0 : t0 + P], ec == 0, ec == EC - 1)
                    pse1 = ppa.tile([P, T], F32, tag="ppa")
                    for ec in range(EC):
                        _mm(nc, pse1[:], QT[:, ec, (2 * b + 1) * P : (2 * b + 2) * P],
                            KT[:, ec, t0 : t0 + T], ec == 0, ec == EC - 1)

                    att0 = esb_p.tile([P, P], F32, tag="esb0")
                    nc.vector.tensor_add(
                        out=att0[:], in0=pse0[:, 0:P], in1=mask_t[:, 0, 0:P]
                    )
                    att1 = esb_p.tile([P, T], F32, tag="esb1")
                    nc.vector.tensor_add(out=att1[:], in0=pse1[:], in1=mask_t[:, 1, :])
                    for att in (att0, att1):
                        nmax = st_p.tile([P, 1], F32, tag="nmax")
                        nc.vector.reduce_max(
                            out=nmax[:], in_=att[:], axis=AX.X, negate=True
                        )
                        den = st_p.tile([P, 1], F32, tag="den")
                        nc.scalar.activation(
                            out=att[:], in_=att[:], func=ACTF.Exp,
                            bias=nmax[:, 0:1], accum_out=den[:],
                        )
                        rec = st_p.tile([P, 1], F32, tag="rec")
                        nc.vector.reciprocal(out=rec[:], in_=den[:])
                        nc.gpsimd.tensor_scalar_mul(
                            out=att[:], in0=att[:], scalar1=rec[:]
                        )

                    # --- transpose att blocks: (i0,j0), (i1,j0), (i1,j1) ---
                    attT = attT_p.tile([P, 2, T], MM_DT, tag="attT")
                    blocks = [(att0, 0, 0, 0), (att1, 0, 0, P), (att1, P, 1, P)]
                    for k, (src, joff, jc, ioff) in enumerate(blocks):
                        tp = ppt.tile([P, P], F32, tag="tp")
                        nc.tensor.transpose(
                            tp[:], src[:, joff : joff + P], ident_t[:]
                        )
                        evac(attT[:, jc, ioff : ioff + P], tp[:], k % 2)

                    # --- AV: o^T[e,i] accumulated over j-chunks ---
                    oTb = ot_p.tile([P, EC, T], MM_DT, tag="oT")
                    for ec in range(EC):
                        po = ppa.tile([P, T], F32, tag="ppa")
                        _mm(nc, po[:], VN[:, 2 * b, ec * P : (ec + 1) * P],
                            attT[:, 0, :], True, False)
                        _mm(nc, po[:, P:T], VN[:, 2 * b + 1, ec * P : (ec + 1) * P],
                            attT[:, 1, P:T], False, True)
                        evac(oTb[:, ec, :], po[:], False)

                    # --- wo partial for this (h, b) ---
                    for tcl in range(2):
                        tcc = 2 * b + tcl
                        pw = ppb.tile([P, E], F32, tag="ppb")
                        for ec in range(EC):
                            _mm(nc, pw[:], oTb[:, ec, tcl * P : (tcl + 1) * P],
                                wo_t[:, ec, :], ec == 0, ec == EC - 1)
                        if h == 0:
                            nc.vector.tensor_copy(attn_acc[:, tcc, :], pw[:])
                        else:
                            nc.vector.tensor_add(
                                out=attn_acc[:, tcc, :],
                                in0=attn_acc[:, tcc, :], in1=pw[:],
                            )
            else:
                # ---- last layer: only the final token of each batch ----
                QTl = qkv_p.tile([P, EC, BPC], MM_DT, tag="qtl")
                for oc in range(EC):
                    ps = ppt.tile([P, BPC], F32, tag="tp")
                    for ec in range(EC):
                        _mm(nc, ps[:], wq_t[:, ec, oc * P : (oc + 1) * P],
                            xT[:, ec, T - 1 :: T], ec == 0, ec == EC - 1)
                    evac(QTl[:, oc, :], ps[:], True)
                oTl = ot_p.tile([P, EC, BPC], MM_DT, tag="oTl")
                for b in range(BPC):
                    t0 = b * T
                    pse = ppa.tile([1, T], F32, tag="ppa")
                    for ec in range(EC):
                        _mm(nc, pse[:], QTl[:, ec, b : b + 1],
                            KT[:, ec, t0 : t0 + T], ec == 0, ec == EC - 1)
                    att = esb_p.tile([1, T], F32, tag="esbl")
                    nmax = st_p.tile([1, 1], F32, tag="nmaxl")
                    nc.vector.reduce_max(
                        out=nmax[:], in_=pse[:], axis=AX.X, negate=True
                    )
                    den = st_p.tile([1, 1], F32, tag="denl")
                    nc.scalar.activation(
                        out=att[:], in_=pse[:], func=ACTF.Exp,
                        bias=nmax[0:1, 0:1], accum_out=den[:],
                    )
                    rec = st_p.tile([1, 1], F32, tag="recl")
                    nc.vector.reciprocal(out=rec[:], in_=den[:])
                    nc.vector.tensor_scalar_mul(out=att[:], in0=att[:], scalar1=rec[:])
                    attTl = attT_p.tile([P, 2, 1], MM_DT, tag="attTl")
                    for jc in range(2):
                        tp = ppt.tile([P, 1], F32, tag="tp")
                        nc.tensor.transpose(
                            tp[:], att[0:1, jc * P : (jc + 1) * P],
                            ident_t[0:1, 0:1],
                        )
                        evac(attTl[:, jc, 0:1], tp[:], jc % 2)
                    for ec in range(EC):
                        po = ppt.tile([P, 1], F32, tag="tp")
                        _mm(nc, po[:], VN[:, 2 * b, ec * P : (ec + 1) * P],
                            attTl[:, 0, 0:1], True, False)
                        _mm(nc, po[:], VN[:, 2 * b + 1, ec * P : (ec + 1) * P],
                            attTl[:, 1, 0:1], False, True)
                        evac(oTl[:, ec, b : b + 1], po[:], False)
                # accumulate wo partials directly in PSUM across (h, ec)
                for ec in range(EC):
                    _mm(nc, pw_last[:], oTl[:, ec, :], wo_t[:, ec, :],
                        h == 0 and ec == 0, h == H - 1 and ec == EC - 1)

        # ---- FFN block ----
        ff1w_t = wtile(wff_p, ff1w[l], "wff1")
        ff2w_t = wtile(wff_p, ff2w[l], "wff2")

        if not last:
            h1N = ff_p.tile([P, TC, E], F32, tag="ff32")
            for tcc in range(TC):
                nc.gpsimd.tensor_add(
                    out=attn_acc[:, tcc, :], in0=attn_acc[:, tcc, :], in1=bo_b[:]
                )
                s1 = tmp_p.tile([P, E], F32, tag="s1")
                nc.gpsimd.tensor_add(
                    out=s1[:], in0=attn_acc[:, tcc, :], in1=xN[:, tcc, :]
                )
                layernorm(s1[:], h1N[:, tcc, :], ln1w_b, ln1b_b, "ln1")

            h1T = ff_p.tile([P, EC, TOK], MM_DT, tag="ffT")
            transpose_into(h1T, h1N)

            r1T = ff_p.tile([P, EC, TOK], MM_DT, tag="ffT")
            for fc in range(EC):
                ps = ppb.tile([P, TOK], F32, tag="ppb")
                for ec in range(EC):
                    _mm(nc, ps[:], ff1w_t[:, ec, fc * P : (fc + 1) * P],
                        h1T[:, ec, :], ec == 0, ec == EC - 1)
                nc.scalar.activation(
                    out=_c(r1T[:, fc, :]), in_=ps[:], func=ACTF.Relu,
                    bias=ff1b_t[:, fc : fc + 1],
                )

            xN_new = act_p.tile([P, TC, E], F32, tag="xN")
            for tcc in range(TC):
                ps = ppb.tile([P, E], F32, tag="ppb")
                for fc in range(EC):
                    _mm(nc, ps[:], r1T[:, fc, tcc * P : (tcc + 1) * P],
                        ff2w_t[:, fc, :], fc == 0, fc == EC - 1)
                s2 = tmp_p.tile([P, E], F32, tag="s1")
                nc.vector.tensor_add(out=s2[:], in0=ps[:], in1=ff2b_b[:])
                nc.gpsimd.tensor_add(out=s2[:], in0=s2[:], in1=attn_acc[:, tcc, :])
                layernorm(s2[:], xN_new[:, tcc, :], ln2w_b, ln2b_b, "ln2")
            xN = xN_new
            xT = act_p.tile([P, EC, TOK], MM_DT, tag="xT")
            transpose_into(xT, xN)
        else:
            # ---- last layer FFN on 2 tokens only ----
            ao_l = out_p.tile([BPC, E], F32)
            nc.vector.tensor_add(out=ao_l[:], in0=pw_last[:], in1=bo_b[:BPC, :])
            x_l = out_p.tile([BPC, E], F32)
            for b in range(BPC):
                nc.sync.dma_start(
                    out=x_l[b : b + 1, :], in_=xN[P - 1 : P, 2 * b + 1, :]
                )
            s1 = out_p.tile([BPC, E], F32)
            nc.vector.tensor_add(out=s1[:], in0=ao_l[:], in1=x_l[:])
            h1_l = out_p.tile([BPC, E], F32)
            layernorm(s1[:], h1_l[:], ln1w_b, ln1b_b, "lnL1", rows=BPC)
            h1T_l = ff_p.tile([P, EC, BPC], MM_DT, tag="h1Tl")
            for bb in range(EC):
                tp = ppt.tile([P, BPC], F32, tag="tp")
                nc.tensor.transpose(
                    tp[:], h1_l[:, bb * P : (bb + 1) * P],
                    ident_t[0:BPC, 0:BPC],
                )
                evac(h1T_l[:, bb, :], tp[:], bb % 2)
            r1T_l = ff_p.tile([P, EC, BPC], MM_DT, tag="r1Tl")
            for fc in range(EC):
                ps = ppt.tile([P, BPC], F32, tag="tp")
                for ec in range(EC):
                    _mm(nc, ps[:], ff1w_t[:, ec, fc * P : (fc + 1) * P],
                        h1T_l[:, ec, :], ec == 0, ec == EC - 1)
                nc.scalar.activation(
                    out=_c(r1T_l[:, fc, :]), in_=ps[:], func=ACTF.Relu,
                    bias=ff1b_t[:, fc : fc + 1],
                )
            ps2 = ppb.tile([BPC, E], F32, tag="ppb")
            for fc in range(EC):
                _mm(nc, ps2[:], r1T_l[:, fc, :], ff2w_t[:, fc, :],
                    fc == 0, fc == EC - 1)
            s2 = out_p.tile([BPC, E], F32)
            nc.vector.tensor_add(out=s2[:], in0=ps2[:], in1=ff2b_b[:BPC, :])
            nc.vector.tensor_add(out=s2[:], in0=s2[:], in1=ao_l[:])
            xl = out_p.tile([BPC, E], F32)
            layernorm(s2[:], xl[:], ln2w_b, ln2b_b, "lnL2", rows=BPC)
            xlT = ff_p.tile([P, EC, BPC], MM_DT, tag="xlT")
            for bb in range(EC):
                tp = ppt.tile([P, BPC], F32, tag="tp")
                nc.tensor.transpose(
                    tp[:], xl[:, bb * P : (bb + 1) * P],
                    ident_t[0:BPC, 0:BPC],
                )
                evac(xlT[:, bb, :], tp[:], bb % 2)

    # ---- output head: last token of each batch ----
    wout_t = wtile(wo_p, wout, "wo")
    bout_t = out_p.tile([BPC, V], F32)
    nc.sync.dma_start(out=bout_t[:], in_=bout.partition_broadcast(BPC))
    pl = ppb.tile([BPC, V], F32, tag="ppb")
    if nlayers == L and last_opt:
        xl_lhs = xlT
        cols = slice(0, BPC)
    else:
        xl_lhs = xT
        cols = slice(T - 1, TOK, T)
    for ec in range(EC):
        _mm(nc, pl[:], xl_lhs[:, ec, cols], wout_t[:, ec, :], ec == 0, ec == EC - 1)
    logits = out_p.tile([BPC, V], F32)
    nc.vector.tensor_add(out=logits[:], in0=pl[:], in1=bout_t[:])
    nmax = out_p.tile([BPC, 1], F32)
    nc.vector.reduce_max(out=nmax[:], in_=logits[:], axis=AX.X, negate=True)
    den = out_p.tile([BPC, 1], F32)
    nc.scalar.activation(
        out=logits[:], in_=logits[:], func=ACTF.Exp,
        bias=nmax[:, 0:1], accum_out=den[:],
    )
    rec = out_p.tile([BPC, 1], F32)
    nc.vector.reciprocal(out=rec[:], in_=den[:])
    nc.vector.tensor_scalar_mul(out=logits[:], in0=logits[:], scalar1=rec[:])
    nc.sync.dma_start(out=probs, in_=logits[:])


def _pe_table():
    i = np.arange(E, dtype=np.float32)
    rates = (1.0 / np.power(np.float32(10000.0), 2.0 * np.floor(i / 2.0) / E)).astype(
        np.float32
    )
    ang = np.arange(T, dtype=np.float32)[:, None] * rates[None, :]
    pe = np.concatenate([np.sin(ang[:, 0::2]), np.cos(ang[:, 1::2])], axis=-1)
    return np.tile(pe.astype(np.float32), (BPC, 1))  # [TOK, E]


def _masks():
    m = np.zeros((2, P, T), dtype=np.float32)
    j = np.arange(T)
    for c in range(2):
        i = c * P + np.arange(P)
        m[c] = np.where(j[None, :] > i[:, None], np.float32(NEG), np.float32(0.0))
    return m


def _prep_in_maps(
    input_tokens, emb, wq, wk, wv, wo, bo, ln1_w, ln1_b, ln2_w, ln2_b,
    ff1_w, ff1_b, ff2_w, ff2_b, wout, bout,
):
    f = lambda x: np.ascontiguousarray(np.asarray(x, dtype=np.float32))
    w = lambda x: np.ascontiguousarray(np.asarray(x, dtype=np.float32).astype(NP_WDT))
    toks = np.asarray(input_tokens).astype(np.int64)
    shared = {
        "emb": f(emb), "wq": w(wq), "wk": w(wk), "wv": w(wv), "wo": w(wo),
        "bo": f(bo), "ln1w": f(ln1_w), "ln1b": f(ln1_b), "ln2w": f(ln2_w),
        "ln2b": f(ln2_b), "ff1w": w(ff1_w), "ff1b": f(ff1_b), "ff2w": w(ff2_w),
        "ff2b": f(ff2_b), "wout": w(wout), "bout": f(bout),
        "pe2": _pe_table(), "masks": _masks(),
        "ident": np.eye(P, dtype=np.float32),
    }
    in_maps = []
    for c in range(NCORES):
        t = toks[c * BPC : (c + 1) * BPC].reshape(TOK)  # [512] flat tokens
        tokarr = np.ascontiguousarray(t.reshape(TC, P).T.astype(np.int32))
        in_maps.append({**shared, "tok": tokarr})
    return in_maps


def kernel(**inputs):
    if "nc" not in _CACHE:
        _CACHE["nc"] = _build()
    nc = _CACHE["nc"]
    in_maps = _prep_in_maps(**inputs)
    res = run_bass_kernel_spmd(nc, in_maps, core_ids=list(range(NCORES)))
    _CACHE["last_results"] = res
    out = np.concatenate([res.results[c]["probs"] for c in range(NCORES)], axis=0)
    return out.astype(np.float32)



# revision 5
# speedup vs baseline: 2.0551x; 2.0551x over previous
"""CheckersGPT dense transformer forward pass on 8 Trainium2 NeuronCores.

Strategy: pure data-parallel over the batch dim (16 batches -> 2 per core).
Each core runs the full 6-layer transformer on its 512 tokens (2 batches x
256 tokens); no collectives, outputs concatenated on the host.

Key restructuring vs a direct translation (each head uses a full ExE Q/K/V):
  - Host folds Wqk = Wq @ Wk^T and Wvo_h = Wv_h @ Wo_h. The K projection and
    the H*E->E output projection disappear: energy = (x Wqk) x^T and
    attn_out = sum_h att_h (x Wvo_h).
  - Energy is computed TRANSPOSED ([j, i] = key-major) with xT as lhsT, so
    att @ V produces attn_out in natural token-major layout directly -- no
    PE transposes of the attention matrix at all.
  - Softmax runs without max-subtraction (energies are bounded ~50 by the
    0.02-scale weights; exp stays finite in fp32). Denominators come from
    N=1 matmuls against a ones vector; 1/den is folded into the PSUM->SBUF
    evacuation of each head's AV product.
  - The head loop is software-pipelined (the b=1 attention tail of head h is
    emitted inside head h+1) so the PE never waits on a softmax chain.

Layout per core (P=128 partitions):
  xT   [128, 4, 512]  : x transposed; chunk c = embed dims [128c,128c+128),
                        free dim = 512 tokens. matmul lhsT/rhs operand.
  xN   [128, 4, 512]  : x natural; chunk c = tokens [128c,128c+128),
                        free dim = 512 embed dims. residuals / layernorm.
All matmuls are out = lhsT.T @ rhs with contraction on the partition dim.
The last layer only computes attention/FFN for the final token of each batch.
"""

import os
import numpy as np
from contextlib import ExitStack

import ml_dtypes
import concourse.bass as bass
import concourse.tile as tile
from concourse import bacc, mybir
from concourse.bass_utils import run_bass_kernel_spmd

F32 = mybir.dt.float32
BF16 = mybir.dt.bfloat16
I32 = mybir.dt.int32
AX = mybir.AxisListType
ALU = mybir.AluOpType
ACTF = mybir.ActivationFunctionType

V, E, L, H, B, T = 512, 512, 6, 8, 16, 256
NCORES = 8
BPC = B // NCORES          # batches per core
TOK = BPC * T              # tokens per core
P = 128
EC = E // P                # embed chunks of 128
TC = TOK // P              # token chunks of 128
NEG = -1e9
EPS = 1e-5

MODE = os.environ.get("CKGPT_MM_DT", "bf16")   # bf16 | f32r | f32
MM_DT = {"bf16": BF16, "f32r": F32, "f32": F32}[MODE]
MM_CAST = mybir.dt.float32r if MODE == "f32r" else None
NP_WDT = ml_dtypes.bfloat16 if MODE == "bf16" else np.float32

_CACHE = {}


def _c(ap):
    """Cast an AP for matmul input (f32r mode only)."""
    return ap.bitcast(MM_CAST) if MM_CAST is not None else ap


def _mm(nc, out, lhsT, rhs, start, stop):
    nc.tensor.matmul(out, _c(lhsT), _c(rhs), start=start, stop=stop)


def _build(nlayers=L, reps=1, last_opt=True):
    nc = bacc.Bacc("TRN2", target_bir_lowering=False, debug=False, num_devices=NCORES)

    def din(name, shape, dtype=F32):
        return nc.dram_tensor(name, list(shape), dtype, kind="ExternalInput").ap()

    tok = din("tok", [P, TC], I32)            # token ids, p-major within chunks
    emb = din("emb", [V, E])
    pe2 = din("pe2", [TOK, E])                # positional encoding tiled over BPC
    wqk = din("wqk", [L, H, E, E], MM_DT)     # Wq @ Wk^T
    wvo = din("wvo", [L, H, E, E], MM_DT)     # Wv @ Wo_h
    bo = din("bo", [L, E])
    ln1w = din("ln1w", [L, E])
    ln1b = din("ln1b", [L, E])
    ln2w = din("ln2w", [L, E])
    ln2b = din("ln2b", [L, E])
    ff1w = din("ff1w", [L, E, E], MM_DT)
    ff1b = din("ff1b", [L, E])
    ff2w = din("ff2w", [L, E, E], MM_DT)
    ff2b = din("ff2b", [L, E])
    wout = din("wout", [E, V], MM_DT)
    bout = din("bout", [V])
    maskd = din("maskd", [P, P])              # additive causal mask, diag block
    ident = din("ident", [P, P])
    probs = nc.dram_tensor("probs", [BPC, V], F32, kind="ExternalOutput").ap()
    aps = (emb, pe2, wqk, wvo, bo, ln1w, ln1b, ln2w, ln2b,
           ff1w, ff1b, ff2w, ff2b, wout, bout, maskd, ident, probs, tok)

    with tile.TileContext(nc) as tc, ExitStack() as ctx:
        if reps > 1:
            with tc.For_i(0, reps, 1):
                _emit(nc, tc, ctx, aps, nlayers, last_opt)
        else:
            _emit(nc, tc, ctx, aps, nlayers, last_opt)

    nc.compile()
    return nc


def _emit(nc, tc, ctx, aps, nlayers, last_opt):
    (emb, pe2, wqk, wvo, bo, ln1w, ln1b, ln2w, ln2b,
     ff1w, ff1b, ff2w, ff2b, wout, bout, maskd, ident, probs, tok) = aps
    ep = ctx.enter_context

    const = ep(tc.tile_pool(name="const", bufs=1))
    wq_p = ep(tc.tile_pool(name="wq", bufs=6))
    wff_p = ep(tc.tile_pool(name="wff", bufs=1))
    bias_p = ep(tc.tile_pool(name="bias", bufs=2))
    act_p = ep(tc.tile_pool(name="act", bufs=2))
    qkv_p = ep(tc.tile_pool(name="qkvact", bufs=2))
    ff_p = ep(tc.tile_pool(name="ffact", bufs=2))
    tmp_p = ep(tc.tile_pool(name="tmp", bufs=4))
    esb_p = ep(tc.tile_pool(name="esb", bufs=4))
    st_p = ep(tc.tile_pool(name="stats", bufs=8))
    out_p = ep(tc.tile_pool(name="outp", bufs=1))

    ppb = ep(tc.tile_pool(name="ppb", bufs=3, space="PSUM"))
    ppa = ep(tc.tile_pool(name="ppa", bufs=2, space="PSUM"))
    ppt = ep(tc.tile_pool(name="ppt", bufs=2, space="PSUM"))
    ppd = ep(tc.tile_pool(name="ppd", bufs=1, space="PSUM"))

    # ---- constants ----
    ident_t = const.tile([P, P], F32)
    nc.sync.dma_start(out=ident_t[:], in_=ident)
    maskd_t = const.tile([P, P], F32)
    nc.sync.dma_start(out=maskd_t[:], in_=maskd)
    eps_t = const.tile([P, 1], F32)
    nc.vector.memset(eps_t[:], EPS)
    ones_t = const.tile([P, 1], MM_DT)
    nc.vector.memset(ones_t[:], 1.0)
    tok_t = const.tile([P, TC], I32)
    nc.sync.dma_start(out=tok_t[:], in_=tok)

    def wtile(pool, dram2d, tag):
        t = pool.tile([P, EC, E], MM_DT, tag=tag)
        nc.sync.dma_start(
            out=_c(t[:]),
            in_=_c(dram2d.rearrange("(c p) o -> p c o", p=P)),
        )
        return t

    def bbcast(vec_ap, tag):
        t = bias_p.tile([P, E], F32, tag=tag)
        nc.sync.dma_start(out=t[:], in_=vec_ap.partition_broadcast(P))
        return t

    def evac(dst, src, use_act):
        """PSUM -> SBUF copy (dtype conversion happens on write)."""
        if use_act:
            nc.scalar.copy(_c(dst), src)
        else:
            nc.vector.tensor_copy(_c(dst), src)

    def layernorm(src, dst, w_b, b_b, tag, rows=P):
        # dst = (src - mean)/sqrt(var+eps) * w + b ; src/dst [rows, E] fp32
        stt = st_p.tile([P, 6], F32, tag=tag + "s")
        nc.vector.bn_stats(out=stt[:rows], in_=src)
        mv = st_p.tile([P, 2], F32, tag=tag + "m")
        nc.vector.bn_aggr(out=mv[:rows], in_=stt[:rows])
        sd = st_p.tile([P, 1], F32, tag=tag + "d")
        nc.scalar.activation(
            out=sd[:rows], in_=mv[:rows, 1:2], func=ACTF.Sqrt, bias=eps_t[:rows, 0:1]
        )
        rs = st_p.tile([P, 1], F32, tag=tag + "r")
        nc.vector.reciprocal(out=rs[:rows], in_=sd[:rows])
        nc.vector.tensor_scalar(
            out=dst, in0=src, scalar1=mv[:rows, 0:1], scalar2=rs[:rows],
            op0=ALU.subtract, op1=ALU.mult,
        )
        nc.gpsimd.tensor_mul(out=dst, in0=dst, in1=w_b[:rows, :])
        nc.gpsimd.tensor_add(out=dst, in0=dst, in1=b_b[:rows, :])

    # ---- embedding gather + positional encoding ----
    xN = act_p.tile([P, TC, E], F32, tag="xN")
    for c in range(TC):
        nc.gpsimd.indirect_dma_start(
            out=xN[:, c, :], out_offset=None, in_=emb,
            in_offset=bass.IndirectOffsetOnAxis(ap=tok_t[:, c : c + 1], axis=0),
        )
    pe_t = ff_p.tile([P, TC, E], F32, tag="ff32")
    nc.sync.dma_start(out=pe_t[:], in_=pe2.rearrange("(c p) o -> p c o", p=P))
    for c in range(TC):
        nc.vector.tensor_add(out=xN[:, c, :], in0=xN[:, c, :], in1=pe_t[:, c, :])
    xT = act_p.tile([P, EC, TOK], MM_DT, tag="xT")
    for a in range(TC):
        for bb in range(EC):
            tp = ppt.tile([P, P], F32, tag="tp")
            nc.tensor.transpose(tp[:], xN[:, a, bb * P : (bb + 1) * P], ident_t[:])
            evac(xT[:, bb, a * P : (a + 1) * P], tp[:], (a + bb) % 2)

    for l in range(nlayers):
        last = last_opt and (l == L - 1) and (nlayers == L)
        biases = {}

        def load_biases(l=l):
            biases["bo"] = bbcast(bo[l], "b_bo")
            biases["ln1w"] = bbcast(ln1w[l], "b_l1w")
            biases["ln1b"] = bbcast(ln1b[l], "b_l1b")
            biases["ln2w"] = bbcast(ln2w[l], "b_l2w")
            biases["ln2b"] = bbcast(ln2b[l], "b_l2b")
            biases["ff2b"] = bbcast(ff2b[l], "b_f2")
            t = bias_p.tile([P, EC], F32, tag="b_f1")
            nc.sync.dma_start(out=t[:], in_=ff1b[l].rearrange("(c p) -> p c", p=P))
            biases["ff1b"] = t

        attn_acc = act_p.tile([P, TC, E], F32, tag="acc")

        def attn_energy(b, GT, xT=xT):
            """Transposed energy + softmax for batch b -> expT [P, 3P] bf16.

            cols [0:2P)  = j-chunk0 rows, i in [0,2P)   (diag block masked)
            cols [2P:3P) = j-chunk1 rows, i in [P,2P)   (diag block masked)
            """
            t0 = b * T
            e0 = ppa.tile([P, T], F32, tag="ppa")
            for ec in range(EC):
                _mm(nc, e0[:], xT[:, ec, t0 : t0 + P],
                    GT[:, ec, t0 : t0 + T], ec == 0, ec == EC - 1)
            e1 = ppa.tile([P, P], F32, tag="ppa")
            for ec in range(EC):
                _mm(nc, e1[:], xT[:, ec, t0 + P : t0 + 2 * P],
                    GT[:, ec, t0 + P : t0 + T], ec == 0, ec == EC - 1)
            nc.vector.tensor_add(out=e0[:, 0:P], in0=e0[:, 0:P], in1=maskd_t[:])
            nc.vector.tensor_add(out=e1[:], in0=e1[:], in1=maskd_t[:])
            expT = esb_p.tile([P, 3 * P], MM_DT, tag="expT")
            nc.scalar.activation(out=_c(expT[:, 0:T]), in_=e0[:], func=ACTF.Exp)
            nc.scalar.activation(out=_c(expT[:, T : T + P]), in_=e1[:], func=ACTF.Exp)
            return expT

        def attn_tail(h, b, expT, VpN, acc):
            """den + AV + normalized accumulation for batch b of head h."""
            den = ppd.tile([P, 2], F32, tag="den")
            _mm(nc, den[:, 0:1], expT[:, 0:P], ones_t[:, 0:1], True, True)
            U0 = ppb.tile([P, E], F32, tag="ppb")
            _mm(nc, U0[:], expT[:, 0:P], VpN[:, 2 * b, :], True, True)
            _mm(nc, den[:, 1:2], expT[:, P : 2 * P], ones_t[:, 0:1], True, False)
            U1 = ppb.tile([P, E], F32, tag="ppb")
            _mm(nc, U1[:], expT[:, P : 2 * P], VpN[:, 2 * b, :], True, False)
            _mm(nc, den[:, 1:2], expT[:, 2 * P : 3 * P], ones_t[:, 0:1], False, True)
            _mm(nc, U1[:], expT[:, 2 * P : 3 * P], VpN[:, 2 * b + 1, :], False, True)
            rec = st_p.tile([P, 2], F32, tag="rec", bufs=3)
            nc.vector.reciprocal(out=rec[:], in_=den[:])
            for ic, U in ((0, U0), (1, U1)):
                tcc = 2 * b + ic
                sc = rec[:, ic : ic + 1]
                if h == 0:
                    if ic == 0:
                        nc.vector.tensor_scalar_mul(
                            out=acc[:, tcc, :], in0=U[:], scalar1=sc
                        )
                    else:
                        nc.scalar.mul(acc[:, tcc, :], U[:], sc)
                else:
                    tmp = tmp_p.tile([P, E], F32, tag="uscale")
                    if ic == 0:
                        nc.vector.tensor_scalar_mul(out=tmp[:], in0=U[:], scalar1=sc)
                    else:
                        nc.scalar.mul(tmp[:], U[:], sc)
                    nc.gpsimd.tensor_add(
                        out=acc[:, tcc, :], in0=acc[:, tcc, :], in1=tmp[:]
                    )

        pending = None
        if not last:
            for h in range(H):
                wqk_t = wtile(wq_p, wqk[l, h], "w")
                wvo_t = wtile(wq_p, wvo[l, h], "w")
                if h == 1:
                    load_biases()
                # G^T = (x @ Wqk)^T, embed-major like the old QT
                GT = qkv_p.tile([P, EC, TOK], MM_DT, tag="gt")
                for oc in range(EC):
                    ps = ppb.tile([P, TOK], F32, tag="ppb")
                    for ec in range(EC):
                        _mm(nc, ps[:], wqk_t[:, ec, oc * P : (oc + 1) * P],
                            xT[:, ec, :], ec == 0, ec == EC - 1)
                    evac(GT[:, oc, :], ps[:], oc % 2)
                # V' = x @ Wvo, token-major
                VpN = qkv_p.tile([P, TC, E], MM_DT, tag="vp", bufs=3)
                for tcc in range(TC):
                    ps = ppb.tile([P, E], F32, tag="ppb")
                    for ec in range(EC):
                        _mm(nc, ps[:], xT[:, ec, tcc * P : (tcc + 1) * P],
                            wvo_t[:, ec, :], ec == 0, ec == EC - 1)
                    evac(VpN[:, tcc, :], ps[:], tcc % 2)
                expT0 = attn_energy(0, GT)
                if pending is not None:
                    pending()
                    pending = None
                attn_tail(h, 0, expT0, VpN, attn_acc)
                expT1 = attn_energy(1, GT)
                pending = (
                    lambda h=h, e=expT1, Vp=VpN: attn_tail(h, 1, e, Vp, attn_acc)
                )
        else:
            # ---- last layer: attention only for the final token per batch ----
            ao_b = [
                out_p.tile([1, E], F32, name=f"ao{b}", tag=f"ao{b}")
                for b in range(BPC)
            ]

            def last_attn(h, b, GTl, VpN):
                t0 = b * T
                pse = ppa.tile([1, T], F32, tag="ppa")
                for ec in range(EC):
                    _mm(nc, pse[:], GTl[:, ec, b : b + 1],
                        xT[:, ec, t0 : t0 + T], ec == 0, ec == EC - 1)
                att = esb_p.tile([1, T], F32, tag="esbl")
                nmax = st_p.tile([1, 1], F32, tag="nmaxl")
                nc.vector.reduce_max(out=nmax[:], in_=pse[:], axis=AX.X, negate=True)
                den = st_p.tile([1, 1], F32, tag="denl")
                nc.scalar.activation(
                    out=att[:], in_=pse[:], func=ACTF.Exp,
                    bias=nmax[0:1, 0:1], accum_out=den[:],
                )
                rec = st_p.tile([1, 1], F32, tag="recl")
                nc.vector.reciprocal(out=rec[:], in_=den[:])
                nc.vector.tensor_scalar_mul(out=att[:], in0=att[:], scalar1=rec[:])
                attTl = esb_p.tile([P, 2, 1], MM_DT, tag="attTl")
                for jc in range(2):
                    tp = ppt.tile([P, 1], F32, tag="tp")
                    nc.tensor.transpose(
                        tp[:], att[0:1, jc * P : (jc + 1) * P], ident_t[0:1, 0:1]
                    )
                    evac(attTl[:, jc, 0:1], tp[:], jc % 2)
                U = ppb.tile([1, E], F32, tag="ppb")
                _mm(nc, U[:], attTl[:, 0, 0:1], VpN[:, 2 * b, :], True, False)
                _mm(nc, U[:], attTl[:, 1, 0:1], VpN[:, 2 * b + 1, :], False, True)
                if h == 0:
                    nc.vector.tensor_copy(ao_b[b][:], U[:])
                else:
                    nc.vector.tensor_add(out=ao_b[b][:], in0=ao_b[b][:], in1=U[:])

            for h in range(H):
                wqk_t = wtile(wq_p, wqk[l, h], "w")
                wvo_t = wtile(wq_p, wvo[l, h], "w")
                if h == 1:
                    load_biases()
                GTl = qkv_p.tile([P, EC, BPC], MM_DT, tag="qtl")
                for oc in range(EC):
                    ps = ppt.tile([P, BPC], F32, tag="tp")
                    for ec in range(EC):
                        _mm(nc, ps[:], wqk_t[:, ec, oc * P : (oc + 1) * P],
                            xT[:, ec, T - 1 :: T], ec == 0, ec == EC - 1)
                    evac(GTl[:, oc, :], ps[:], oc % 2)
                VpN = qkv_p.tile([P, TC, E], MM_DT, tag="vp", bufs=3)
                for tcc in range(TC):
                    ps = ppb.tile([P, E], F32, tag="ppb")
                    for ec in range(EC):
                        _mm(nc, ps[:], xT[:, ec, tcc * P : (tcc + 1) * P],
                            wvo_t[:, ec, :], ec == 0, ec == EC - 1)
                    evac(VpN[:, tcc, :], ps[:], tcc % 2)
                if pending is not None:
                    pending()
                pending = (
                    lambda h=h, G=GTl, Vp=VpN: [last_attn(h, b, G, Vp)
                                                for b in range(BPC)]
                )
        if pending is not None:
            pending()
            pending = None

        # ---- FFN block ----
        ff1w_t = wtile(wff_p, ff1w[l], "wff1")
        ff2w_t = wtile(wff_p, ff2w[l], "wff2")

        if not last:
            h1N = ff_p.tile([P, TC, E], F32, tag="ff32")
            h1T = ff_p.tile([P, EC, TOK], MM_DT, tag="ffT")
            for tcc in range(TC):
                nc.gpsimd.tensor_add(
                    out=attn_acc[:, tcc, :], in0=attn_acc[:, tcc, :],
                    in1=biases["bo"][:],
                )
                s1 = tmp_p.tile([P, E], F32, tag="s1")
                nc.gpsimd.tensor_add(
                    out=s1[:], in0=attn_acc[:, tcc, :], in1=xN[:, tcc, :]
                )
                layernorm(s1[:], h1N[:, tcc, :], biases["ln1w"], biases["ln1b"],
                          "ln1")
                for bb in range(EC):
                    tp = ppt.tile([P, P], F32, tag="tp")
                    nc.tensor.transpose(
                        tp[:], h1N[:, tcc, bb * P : (bb + 1) * P], ident_t[:]
                    )
                    evac(h1T[:, bb, tcc * P : (tcc + 1) * P], tp[:], (tcc + bb) % 2)

            r1T = ff_p.tile([P, EC, TOK], MM_DT, tag="ffT")
            for fc in range(EC):
                ps = ppb.tile([P, TOK], F32, tag="ppb")
                for ec in range(EC):
                    _mm(nc, ps[:], ff1w_t[:, ec, fc * P : (fc + 1) * P],
                        h1T[:, ec, :], ec == 0, ec == EC - 1)
                nc.scalar.activation(
                    out=_c(r1T[:, fc, :]), in_=ps[:], func=ACTF.Relu,
                    bias=biases["ff1b"][:, fc : fc + 1],
                )

            xN_new = act_p.tile([P, TC, E], F32, tag="xN")
            xT_new = act_p.tile([P, EC, TOK], MM_DT, tag="xT")
            for tcc in range(TC):
                ps = ppb.tile([P, E], F32, tag="ppb")
                for fc in range(EC):
                    _mm(nc, ps[:], r1T[:, fc, tcc * P : (tcc + 1) * P],
                        ff2w_t[:, fc, :], fc == 0, fc == EC - 1)
                s2 = tmp_p.tile([P, E], F32, tag="s1")
                nc.vector.tensor_add(out=s2[:], in0=ps[:], in1=biases["ff2b"][:])
                nc.gpsimd.tensor_add(
                    out=s2[:], in0=s2[:], in1=attn_acc[:, tcc, :]
                )
                layernorm(s2[:], xN_new[:, tcc, :], biases["ln2w"], biases["ln2b"],
                          "ln2")
                for bb in range(EC):
                    tp = ppt.tile([P, P], F32, tag="tp")
                    nc.tensor.transpose(
                        tp[:], xN_new[:, tcc, bb * P : (bb + 1) * P], ident_t[:]
                    )
                    evac(xT_new[:, bb, tcc * P : (tcc + 1) * P], tp[:],
                         (tcc + bb) % 2)
            xN = xN_new
            xT = xT_new
        else:
            # ---- last layer FFN on the 2 final tokens only ----
            ao_l = out_p.tile([BPC, E], F32)
            for b in range(BPC):
                nc.sync.dma_start(out=ao_l[b : b + 1, :], in_=ao_b[b][:])
            nc.vector.tensor_add(
                out=ao_l[:], in0=ao_l[:], in1=biases["bo"][:BPC, :]
            )
            x_l = out_p.tile([BPC, E], F32)
            for b in range(BPC):
                nc.sync.dma_start(
                    out=x_l[b : b + 1, :], in_=xN[P - 1 : P, 2 * b + 1, :]
                )
            s1 = out_p.tile([BPC, E], F32)
            nc.vector.tensor_add(out=s1[:], in0=ao_l[:], in1=x_l[:])
            h1_l = out_p.tile([BPC, E], F32)
            layernorm(s1[:], h1_l[:], biases["ln1w"], biases["ln1b"], "lnL1",
                      rows=BPC)
            h1T_l = ff_p.tile([P, EC, BPC], MM_DT, tag="h1Tl")
            for bb in range(EC):
                tp = ppt.tile([P, BPC], F32, tag="tp")
                nc.tensor.transpose(
                    tp[:], h1_l[:, bb * P : (bb + 1) * P], ident_t[0:BPC, 0:BPC]
                )
                evac(h1T_l[:, bb, :], tp[:], bb % 2)
            r1T_l = ff_p.tile([P, EC, BPC], MM_DT, tag="r1Tl")
            for fc in range(EC):
                ps = ppt.tile([P, BPC], F32, tag="tp")
                for ec in range(EC):
                    _mm(nc, ps[:], ff1w_t[:, ec, fc * P : (fc + 1) * P],
                        h1T_l[:, ec, :], ec == 0, ec == EC - 1)
                nc.scalar.activation(
                    out=_c(r1T_l[:, fc, :]), in_=ps[:], func=ACTF.Relu,
                    bias=biases["ff1b"][:, fc : fc + 1],
                )
            ps2 = ppb.tile([BPC, E], F32, tag="ppb")
            for fc in range(EC):
                _mm(nc, ps2[:], r1T_l[:, fc, :], ff2w_t[:, fc, :],
                    fc == 0, fc == EC - 1)
            s2 = out_p.tile([BPC, E], F32)
            nc.vector.tensor_add(out=s2[:], in0=ps2[:], in1=biases["ff2b"][:BPC, :])
            nc.vector.tensor_add(out=s2[:], in0=s2[:], in1=ao_l[:])
            xl = out_p.tile([BPC, E], F32)
            layernorm(s2[:], xl[:], biases["ln2w"], biases["ln2b"], "lnL2",
                      rows=BPC)
            xlT = ff_p.tile([P, EC, BPC], MM_DT, tag="xlT")
            for bb in range(EC):
                tp = ppt.tile([P, BPC], F32, tag="tp")
                nc.tensor.transpose(
                    tp[:], xl[:, bb * P : (bb + 1) * P], ident_t[0:BPC, 0:BPC]
                )
                evac(xlT[:, bb, :], tp[:], bb % 2)

    # ---- output head: last token of each batch ----
    wout_t = wff_p.tile([P, EC, V], MM_DT, tag="wout")
    nc.sync.dma_start(
        out=_c(wout_t[:]), in_=_c(wout.rearrange("(c p) o -> p c o", p=P))
    )
    bout_t = out_p.tile([BPC, V], F32)
    nc.sync.dma_start(out=bout_t[:], in_=bout.partition_broadcast(BPC))
    pl = ppb.tile([BPC, V], F32, tag="ppb")
    if nlayers == L and last_opt:
        xl_lhs = xlT
        cols = slice(0, BPC)
    else:
        xl_lhs = xT
        cols = slice(T - 1, TOK, T)
    for ec in range(EC):
        _mm(nc, pl[:], xl_lhs[:, ec, cols], wout_t[:, ec, :], ec == 0, ec == EC - 1)
    logits = out_p.tile([BPC, V], F32)
    nc.vector.tensor_add(out=logits[:], in0=pl[:], in1=bout_t[:])
    nmax = out_p.tile([BPC, 1], F32)
    nc.vector.reduce_max(out=nmax[:], in_=logits[:], axis=AX.X, negate=True)
    den = out_p.tile([BPC, 1], F32)
    nc.scalar.activation(
        out=logits[:], in_=logits[:], func=ACTF.Exp,
        bias=nmax[:, 0:1], accum_out=den[:],
    )
    rec = out_p.tile([BPC, 1], F32)
    nc.vector.reciprocal(out=rec[:], in_=den[:])
    nc.vector.tensor_scalar_mul(out=logits[:], in0=logits[:], scalar1=rec[:])
    nc.sync.dma_start(out=probs, in_=logits[:])


def _pe_table():
    i = np.arange(E, dtype=np.float32)
    rates = (1.0 / np.power(np.float32(10000.0), 2.0 * np.floor(i / 2.0) / E)).astype(
        np.float32
    )
    ang = np.arange(T, dtype=np.float32)[:, None] * rates[None, :]
    pe = np.concatenate([np.sin(ang[:, 0::2]), np.cos(ang[:, 1::2])], axis=-1)
    return np.tile(pe.astype(np.float32), (BPC, 1))  # [TOK, E]


def _maskd():
    j = np.arange(P)
    return np.where(j[None, :] < j[:, None], np.float32(NEG), np.float32(0.0))


def _prep_in_maps(
    input_tokens, emb, wq, wk, wv, wo, bo, ln1_w, ln1_b, ln2_w, ln2_b,
    ff1_w, ff1_b, ff2_w, ff2_b, wout, bout,
):
    f = lambda x: np.ascontiguousarray(np.asarray(x, dtype=np.float32))
    w = lambda x: np.ascontiguousarray(np.asarray(x, dtype=np.float32).astype(NP_WDT))
    wq_ = np.asarray(wq, dtype=np.float32)
    wk_ = np.asarray(wk, dtype=np.float32)
    wv_ = np.asarray(wv, dtype=np.float32)
    wo_ = np.asarray(wo, dtype=np.float32).reshape(L, H, E, E)
    wqk = np.matmul(wq_, np.swapaxes(wk_, -1, -2))
    wvo = np.matmul(wv_, wo_)
    toks = np.asarray(input_tokens).astype(np.int64)
    shared = {
        "emb": f(emb), "wqk": w(wqk), "wvo": w(wvo),
        "bo": f(bo), "ln1w": f(ln1_w), "ln1b": f(ln1_b), "ln2w": f(ln2_w),
        "ln2b": f(ln2_b), "ff1w": w(ff1_w), "ff1b": f(ff1_b), "ff2w": w(ff2_w),
        "ff2b": f(ff2_b), "wout": w(wout), "bout": f(bout),
        "pe2": _pe_table(), "maskd": _maskd(),
        "ident": np.eye(P, dtype=np.float32),
    }
    in_maps = []
    for c in range(NCORES):
        t = toks[c * BPC : (c + 1) * BPC].reshape(TOK)  # [512] flat tokens
        tokarr = np.ascontiguousarray(t.reshape(TC, P).T.astype(np.int32))
        in_maps.append({**shared, "tok": tokarr})
    return in_maps


def kernel(**inputs):
    if "nc" not in _CACHE:
        _CACHE["nc"] = _build()
    nc = _CACHE["nc"]
    in_maps = _prep_in_maps(**inputs)
    res = run_bass_kernel_spmd(nc, in_maps, core_ids=list(range(NCORES)))
    _CACHE["last_results"] = res
    out = np.concatenate([res.results[c]["probs"] for c in range(NCORES)], axis=0)
    return out.astype(np.float32)


# revision 33
# speedup vs baseline: 2.1737x; 1.0577x over previous
"""CheckersGPT dense transformer forward pass on 8 Trainium2 NeuronCores.

Strategy: pure data-parallel over the batch dim (16 batches -> 2 per core).
Each core runs the full 6-layer transformer on its 512 tokens (2 batches x
256 tokens); no collectives, outputs concatenated on the host.

Key restructuring vs a direct translation (each head uses a full ExE Q/K/V):
  - Host folds Wqk = Wq @ Wk^T and Wvo_h = Wv_h @ Wo_h. The K projection and
    the H*E->E output projection disappear: energy = (x Wqk) x^T and
    attn_out = sum_h att_h (x Wvo_h).
  - Energy is computed TRANSPOSED ([j, i] = key-major) with xT as lhsT, so
    att @ V produces attn_out in natural token-major layout directly -- no
    PE transposes of the attention matrix at all.
  - Softmax runs without max-subtraction (energies are bounded ~50 by the
    0.02-scale weights; exp stays finite in fp32). Denominators come from
    N=1 matmuls against a ones vector; 1/den is folded into the PSUM->SBUF
    evacuation of each head's AV product.
  - The head loop is software-pipelined (the b=1 attention tail of head h is
    emitted inside head h+1) so the PE never waits on a softmax chain.

Layout per core (P=128 partitions):
  xT   [128, 4, 512]  : x transposed; chunk c = embed dims [128c,128c+128),
                        free dim = 512 tokens. matmul lhsT/rhs operand.
  xN   [128, 4, 512]  : x natural; chunk c = tokens [128c,128c+128),
                        free dim = 512 embed dims. residuals / layernorm.
All matmuls are out = lhsT.T @ rhs with contraction on the partition dim.
The last layer only computes attention/FFN for the final token of each batch.
"""

import os
import numpy as np
from contextlib import ExitStack

import ml_dtypes
import concourse.bass as bass
import concourse.tile as tile
from concourse import bacc, mybir
from concourse.bass_utils import run_bass_kernel_spmd

F32 = mybir.dt.float32
BF16 = mybir.dt.bfloat16
I32 = mybir.dt.int32
AX = mybir.AxisListType
ALU = mybir.AluOpType
ACTF = mybir.ActivationFunctionType

V, E, L, H, B, T = 512, 512, 6, 8, 16, 256
NCORES = 8
BPC = B // NCORES          # batches per core
TOK = BPC * T              # tokens per core
P = 128
EC = E // P                # embed chunks of 128
TC = TOK // P              # token chunks of 128
NEG = -1e9
EPS = 1e-5

MODE = os.environ.get("CKGPT_MM_DT", "bf16")   # bf16 | f32r | f32
MM_DT = {"bf16": BF16, "f32r": F32, "f32": F32}[MODE]
MM_CAST = mybir.dt.float32r if MODE == "f32r" else None
NP_WDT = ml_dtypes.bfloat16 if MODE == "bf16" else np.float32

_CACHE = {}


def _c(ap):
    """Cast an AP for matmul input (f32r mode only)."""
    return ap.bitcast(MM_CAST) if MM_CAST is not None else ap


def _mm(nc, out, lhsT, rhs, start, stop):
    nc.tensor.matmul(out, _c(lhsT), _c(rhs), start=start, stop=stop)


def _build(nlayers=L, reps=1, last_opt=True):
    nc = bacc.Bacc("TRN2", target_bir_lowering=False, debug=False, num_devices=NCORES)

    def din(name, shape, dtype=F32):
        return nc.dram_tensor(name, list(shape), dtype, kind="ExternalInput").ap()

    tok = din("tok", [P, TC], I32)            # token ids, p-major within chunks
    emb = din("emb", [V, E])
    pe2 = din("pe2", [TOK, E])                # positional encoding tiled over BPC
    wqk = din("wqk", [L, H, E, E], MM_DT)     # Wq @ Wk^T
    wvo = din("wvo", [L, H, E, E], MM_DT)     # Wv @ Wo_h
    bo = din("bo", [L, E])
    ln2w = din("ln2w", [L, E])
    ln2b = din("ln2b", [L, E])
    ff1w = din("ff1w", [L, E, E], MM_DT)    # ln1_w/ln1_b pre-folded on host
    ff1b = din("ff1b", [L, E])
    ff2w = din("ff2w", [L, E, E], MM_DT)
    ff2b = din("ff2b", [L, E])
    wout = din("wout", [E, V], MM_DT)
    bout = din("bout", [V])
    maskd = din("maskd", [P, P])              # additive causal mask, diag block
    ident = din("ident", [P, P])
    probs = nc.dram_tensor("probs", [BPC, V], F32, kind="ExternalOutput").ap()
    aps = (emb, pe2, wqk, wvo, bo, ln2w, ln2b,
           ff1w, ff1b, ff2w, ff2b, wout, bout, maskd, ident, probs, tok)

    with tile.TileContext(nc) as tc, ExitStack() as ctx:
        if reps > 1:
            with tc.For_i(0, reps, 1):
                _emit(nc, tc, ctx, aps, nlayers, last_opt)
        else:
            _emit(nc, tc, ctx, aps, nlayers, last_opt)

    nc.compile()
    return nc


def _emit(nc, tc, ctx, aps, nlayers, last_opt):
    (emb, pe2, wqk, wvo, bo, ln2w, ln2b,
     ff1w, ff1b, ff2w, ff2b, wout, bout, maskd, ident, probs, tok) = aps
    ep = ctx.enter_context

    const = ep(tc.tile_pool(name="const", bufs=1))
    wq_p = ep(tc.tile_pool(name="wq", bufs=8))
    wff_p = ep(tc.tile_pool(name="wff", bufs=1))
    bias_p = ep(tc.tile_pool(name="bias", bufs=2))
    act_p = ep(tc.tile_pool(name="act", bufs=2))
    qkv_p = ep(tc.tile_pool(name="qkvact", bufs=2))
    ff_p = ep(tc.tile_pool(name="ffact", bufs=2))
    tmp_p = ep(tc.tile_pool(name="tmp", bufs=4))
    esb_p = ep(tc.tile_pool(name="esb", bufs=4))
    st_p = ep(tc.tile_pool(name="stats", bufs=8))
    out_p = ep(tc.tile_pool(name="outp", bufs=1))

    ppb = ep(tc.tile_pool(name="ppb", bufs=3, space="PSUM"))
    ppa = ep(tc.tile_pool(name="ppa", bufs=2, space="PSUM"))
    ppt = ep(tc.tile_pool(name="ppt", bufs=2, space="PSUM"))
    ppd = ep(tc.tile_pool(name="ppd", bufs=1, space="PSUM"))

    # ---- constants (tok first: the embedding gather gates the whole start) --
    tok_t = const.tile([P, TC], I32)
    nc.sync.dma_start(out=tok_t[:], in_=tok)
    ident_t = const.tile([P, P], F32)
    nc.sync.dma_start(out=ident_t[:], in_=ident)
    maskd_t = const.tile([P, P], F32)
    nc.sync.dma_start(out=maskd_t[:], in_=maskd)
    eps_t = const.tile([P, 1], F32)
    nc.vector.memset(eps_t[:], EPS)
    ones_t = const.tile([P, 1], MM_DT)
    nc.vector.memset(ones_t[:], 1.0)
    def wtile(pool, dram2d, tag):
        t = pool.tile([P, EC, E], MM_DT, tag=tag)
        nc.sync.dma_start(
            out=_c(t[:]),
            in_=_c(dram2d.rearrange("(c p) o -> p c o", p=P)),
        )
        return t

    def bbcast(vec_ap, tag):
        t = bias_p.tile([P, E], F32, tag=tag)
        nc.sync.dma_start(out=t[:], in_=vec_ap.partition_broadcast(P))
        return t

    def evac(dst, src, use_act):
        """PSUM -> SBUF copy (dtype conversion happens on write)."""
        if use_act:
            nc.scalar.copy(_c(dst), src)
        else:
            nc.vector.tensor_copy(_c(dst), src)

    def layernorm(src, dst, w_b, b_b, tag, rows=P, ts_pool=False):
        # dst = (src - mean)/sqrt(var+eps) [* w + b] ; src/dst [rows, E] fp32
        stt = st_p.tile([P, 6], F32, tag=tag + "s")
        nc.vector.bn_stats(out=stt[:rows], in_=src)
        mv = st_p.tile([P, 2], F32, tag=tag + "m")
        nc.vector.bn_aggr(out=mv[:rows], in_=stt[:rows])
        sd = st_p.tile([P, 1], F32, tag=tag + "d")
        nc.scalar.activation(
            out=sd[:rows], in_=mv[:rows, 1:2], func=ACTF.Sqrt, bias=eps_t[:rows, 0:1]
        )
        rs = st_p.tile([P, 1], F32, tag=tag + "r")
        nc.vector.reciprocal(out=rs[:rows], in_=sd[:rows])
        eng = nc.gpsimd if ts_pool else nc.vector
        eng.tensor_scalar(
            out=dst, in0=src, scalar1=mv[:rows, 0:1], scalar2=rs[:rows],
            op0=ALU.subtract, op1=ALU.mult,
        )
        if w_b is not None:
            nc.gpsimd.tensor_mul(out=dst, in0=dst, in1=w_b[:rows, :])
            nc.gpsimd.tensor_add(out=dst, in0=dst, in1=b_b[:rows, :])

    # ---- embedding gather + positional encoding ----
    xN = act_p.tile([P, TC, E], F32, tag="xN")
    for c in range(TC):
        nc.gpsimd.indirect_dma_start(
            out=xN[:, c, :], out_offset=None, in_=emb,
            in_offset=bass.IndirectOffsetOnAxis(ap=tok_t[:, c : c + 1], axis=0),
        )
    pe_t = ff_p.tile([P, TC, E], F32, tag="ff32")
    nc.sync.dma_start(out=pe_t[:], in_=pe2.rearrange("(c p) o -> p c o", p=P))
    for c in range(TC):
        nc.vector.tensor_add(out=xN[:, c, :], in0=xN[:, c, :], in1=pe_t[:, c, :])
    xT = act_p.tile([P, EC, TOK], MM_DT, tag="xT")
    for a in range(TC):
        for bb in range(EC):
            tp = ppt.tile([P, P], F32, tag="tp")
            nc.tensor.transpose(tp[:], xN[:, a, bb * P : (bb + 1) * P], ident_t[:])
            evac(xT[:, bb, a * P : (a + 1) * P], tp[:], (a + bb) % 2)

    for l in range(nlayers):
        last = last_opt and (l == L - 1) and (nlayers == L)
        biases = {}

        def load_biases(stage, l=l):
            # staggered so bias DMAs don't jam the queue ahead of head weights
            if stage == 1:
                biases["bo"] = bbcast(bo[l], "b_bo")
                # bo folded into the residual source: xN is only read by s1
                for tcc in range(TC):
                    nc.gpsimd.tensor_add(
                        out=xN[:, tcc, :], in0=xN[:, tcc, :], in1=biases["bo"][:]
                    )
            elif stage == 2:
                biases["ln2w"] = bbcast(ln2w[l], "b_l2w")
                biases["ln2b"] = bbcast(ln2b[l], "b_l2b")
            else:
                biases["ff2b"] = bbcast(ff2b[l], "b_f2")  # host: ff2b + bo
                t = bias_p.tile([P, EC], F32, tag="b_f1")
                nc.sync.dma_start(
                    out=t[:], in_=ff1b[l].rearrange("(c p) -> p c", p=P)
                )
                biases["ff1b"] = t

        attn_acc = act_p.tile([P, TC, E], F32, tag="acc")

        def attn_energy(b, GT, xT=xT):
            """Transposed energy + softmax for batch b -> expT [P, 3P] bf16.

            cols [0:2P)  = j-chunk0 rows, i in [0,2P)   (diag block masked)
            cols [2P:3P) = j-chunk1 rows, i in [P,2P)   (diag block masked)
            """
            t0 = b * T
            e0 = ppa.tile([P, T], F32, tag="ppa")
            for ec in range(EC):
                _mm(nc, e0[:], xT[:, ec, t0 : t0 + P],
                    GT[:, ec, t0 : t0 + T], ec == 0, ec == EC - 1)
            e1 = ppa.tile([P, P], F32, tag="ppa")
            for ec in range(EC):
                _mm(nc, e1[:], xT[:, ec, t0 + P : t0 + 2 * P],
                    GT[:, ec, t0 + P : t0 + T], ec == 0, ec == EC - 1)
            nc.vector.tensor_add(out=e0[:, 0:P], in0=e0[:, 0:P], in1=maskd_t[:])
            nc.vector.tensor_add(out=e1[:], in0=e1[:], in1=maskd_t[:])
            expT = esb_p.tile([P, 3 * P], MM_DT, tag="expT")
            nc.scalar.activation(out=_c(expT[:, 0:T]), in_=e0[:], func=ACTF.Exp)
            nc.scalar.activation(out=_c(expT[:, T : T + P]), in_=e1[:], func=ACTF.Exp)
            return expT

        def attn_tail(h, b, expT, VpN, acc):
            """den + AV + normalized accumulation for batch b of head h."""
            den = ppd.tile([P, 2], F32, tag="den")
            _mm(nc, den[:, 0:1], expT[:, 0:P], ones_t[:, 0:1], True, True)
            U0 = ppb.tile([P, E], F32, tag="ppb")
            _mm(nc, U0[:], expT[:, 0:P], VpN[:, 2 * b, :], True, True)
            _mm(nc, den[:, 1:2], expT[:, P : 2 * P], ones_t[:, 0:1], True, False)
            U1 = ppb.tile([P, E], F32, tag="ppb")
            _mm(nc, U1[:], expT[:, P : 2 * P], VpN[:, 2 * b, :], True, False)
            _mm(nc, den[:, 1:2], expT[:, 2 * P : 3 * P], ones_t[:, 0:1], False, True)
            _mm(nc, U1[:], expT[:, 2 * P : 3 * P], VpN[:, 2 * b + 1, :], False, True)
            rec = st_p.tile([P, 2], F32, tag="rec", bufs=3)
            nc.vector.reciprocal(out=rec[:], in_=den[:])
            for ic, U in ((0, U0), (1, U1)):
                tcc = 2 * b + ic
                sc = rec[:, ic : ic + 1]
                if h == 0:
                    if ic == 0:
                        nc.vector.tensor_scalar_mul(
                            out=acc[:, tcc, :], in0=U[:], scalar1=sc
                        )
                    else:
                        nc.scalar.mul(acc[:, tcc, :], U[:], sc)
                else:
                    tmp = tmp_p.tile([P, E], F32, tag="uscale")
                    if ic == 0:
                        nc.vector.tensor_scalar_mul(out=tmp[:], in0=U[:], scalar1=sc)
                    else:
                        nc.scalar.mul(tmp[:], U[:], sc)
                    nc.gpsimd.tensor_add(
                        out=acc[:, tcc, :], in0=acc[:, tcc, :], in1=tmp[:]
                    )

        pending = None
        if not last:
            for h in range(H):
                wqk_t = wtile(wq_p, wqk[l, h], "w")
                wvo_t = wtile(wq_p, wvo[l, h], "w")
                if h in (1, 2, 3):
                    load_biases(h)
                # G^T = (x @ Wqk)^T, embed-major like the old QT
                GT = qkv_p.tile([P, EC, TOK], MM_DT, tag="gt")
                for oc in range(EC):
                    ps = ppb.tile([P, TOK], F32, tag="ppb")
                    for ec in range(EC):
                        _mm(nc, ps[:], wqk_t[:, ec, oc * P : (oc + 1) * P],
                            xT[:, ec, :], ec == 0, ec == EC - 1)
                    evac(GT[:, oc, :], ps[:], oc % 2)
                # V' = x @ Wvo, token-major
                VpN = qkv_p.tile([P, TC, E], MM_DT, tag="vp", bufs=3)
                for tcc in range(TC):
                    ps = ppb.tile([P, E], F32, tag="ppb")
                    for ec in range(EC):
                        _mm(nc, ps[:], xT[:, ec, tcc * P : (tcc + 1) * P],
                            wvo_t[:, ec, :], ec == 0, ec == EC - 1)
                    evac(VpN[:, tcc, :], ps[:], tcc % 2)
                expT0 = attn_energy(0, GT)
                if pending is not None:
                    pending()
                    pending = None
                attn_tail(h, 0, expT0, VpN, attn_acc)
                expT1 = attn_energy(1, GT)
                pending = (
                    lambda h=h, e=expT1, Vp=VpN: attn_tail(h, 1, e, Vp, attn_acc)
                )
        else:
            # ---- last layer: attention only for the final token per batch ----
            ao_b = [
                out_p.tile([1, E], F32, name=f"ao{b}", tag=f"ao{b}")
                for b in range(BPC)
            ]

            def last_attn(h, b, GTl, VpN):
                t0 = b * T
                pse = ppa.tile([1, T], F32, tag="ppa")
                for ec in range(EC):
                    _mm(nc, pse[:], GTl[:, ec, b : b + 1],
                        xT[:, ec, t0 : t0 + T], ec == 0, ec == EC - 1)
                att = esb_p.tile([1, T], F32, tag="esbl")
                nmax = st_p.tile([1, 1], F32, tag="nmaxl")
                nc.vector.reduce_max(out=nmax[:], in_=pse[:], axis=AX.X, negate=True)
                den = st_p.tile([1, 1], F32, tag="denl")
                nc.scalar.activation(
                    out=att[:], in_=pse[:], func=ACTF.Exp,
                    bias=nmax[0:1, 0:1], accum_out=den[:],
                )
                rec = st_p.tile([1, 1], F32, tag="recl")
                nc.vector.reciprocal(out=rec[:], in_=den[:])
                nc.vector.tensor_scalar_mul(out=att[:], in0=att[:], scalar1=rec[:])
                attTl = esb_p.tile([P, 2, 1], MM_DT, tag="attTl")
                for jc in range(2):
                    tp = ppt.tile([P, 1], F32, tag="tp")
                    nc.tensor.transpose(
                        tp[:], att[0:1, jc * P : (jc + 1) * P], ident_t[0:1, 0:1]
                    )
                    evac(attTl[:, jc, 0:1], tp[:], jc % 2)
                U = ppb.tile([1, E], F32, tag="ppb")
                _mm(nc, U[:], attTl[:, 0, 0:1], VpN[:, 2 * b, :], True, False)
                _mm(nc, U[:], attTl[:, 1, 0:1], VpN[:, 2 * b + 1, :], False, True)
                if h == 0:
                    nc.vector.tensor_copy(ao_b[b][:], U[:])
                else:
                    nc.vector.tensor_add(out=ao_b[b][:], in0=ao_b[b][:], in1=U[:])

            for h in range(H):
                wqk_t = wtile(wq_p, wqk[l, h], "w")
                wvo_t = wtile(wq_p, wvo[l, h], "w")
                if h in (1, 2, 3):
                    load_biases(h)
                GTl = qkv_p.tile([P, EC, BPC], MM_DT, tag="qtl")
                for oc in range(EC):
                    ps = ppt.tile([P, BPC], F32, tag="tp")
                    for ec in range(EC):
                        _mm(nc, ps[:], wqk_t[:, ec, oc * P : (oc + 1) * P],
                            xT[:, ec, T - 1 :: T], ec == 0, ec == EC - 1)
                    evac(GTl[:, oc, :], ps[:], oc % 2)
                VpN = qkv_p.tile([P, TC, E], MM_DT, tag="vp", bufs=3)
                for tcc in range(TC):
                    ps = ppb.tile([P, E], F32, tag="ppb")
                    for ec in range(EC):
                        _mm(nc, ps[:], xT[:, ec, tcc * P : (tcc + 1) * P],
                            wvo_t[:, ec, :], ec == 0, ec == EC - 1)
                    evac(VpN[:, tcc, :], ps[:], tcc % 2)
                if pending is not None:
                    pending()
                pending = (
                    lambda h=h, G=GTl, Vp=VpN: [last_attn(h, b, G, Vp)
                                                for b in range(BPC)]
                )
        # ---- FFN block ----
        ff1w_t = wtile(wff_p, ff1w[l], "wff1")
        ff2w_t = wtile(wff_p, ff2w[l], "wff2")

        if not last:
            h1N = ff_p.tile([P, TC, E], F32, tag="ff32")
            h1T = ff_p.tile([P, EC, TOK], MM_DT, tag="ffT")
            r1T = ff_p.tile([P, EC, TOK], MM_DT, tag="ffT")
            xN_new = act_p.tile([P, TC, E], F32, tag="xN")
            xT_new = act_p.tile([P, EC, TOK], MM_DT, tag="xT")

            def ffn_pre(tcc):
                # residual + LN1 core (LN1 w/b folded into ff1; bo into xN/ff2b)
                s1 = tmp_p.tile([P, E], F32, tag="s1")
                nc.gpsimd.tensor_add(
                    out=s1[:], in0=attn_acc[:, tcc, :], in1=xN[:, tcc, :]
                )
                layernorm(s1[:], h1N[:, tcc, :], None, None, "ln1",
                          ts_pool=False)
                for bb in range(EC):
                    tp = ppt.tile([P, P], F32, tag="tp")
                    nc.tensor.transpose(
                        tp[:], h1N[:, tcc, bb * P : (bb + 1) * P], ident_t[:]
                    )
                    evac(h1T[:, bb, tcc * P : (tcc + 1) * P], tp[:], bb % 4 != 0)

            def ffn_mid(half):
                # ff1 + relu on a 256-token half (pipelines with LN1 chunks)
                c0 = half * T
                for fc in range(EC):
                    ps = ppb.tile([P, T], F32, tag="ppb")
                    for ec in range(EC):
                        _mm(nc, ps[:], ff1w_t[:, ec, fc * P : (fc + 1) * P],
                            h1T[:, ec, c0 : c0 + T], ec == 0, ec == EC - 1)
                    nc.scalar.activation(
                        out=_c(r1T[:, fc, c0 : c0 + T]), in_=ps[:], func=ACTF.Relu,
                        bias=biases["ff1b"][:, fc : fc + 1],
                    )

            def ffn_post(tcc):
                ps = ppb.tile([P, E], F32, tag="ppb")
                for fc in range(EC):
                    _mm(nc, ps[:], r1T[:, fc, tcc * P : (tcc + 1) * P],
                        ff2w_t[:, fc, :], fc == 0, fc == EC - 1)
                s2 = tmp_p.tile([P, E], F32, tag="s1")
                nc.vector.tensor_add(out=s2[:], in0=ps[:], in1=biases["ff2b"][:])
                nc.gpsimd.tensor_add(out=s2[:], in0=s2[:], in1=attn_acc[:, tcc, :])
                layernorm(s2[:], xN_new[:, tcc, :], biases["ln2w"], biases["ln2b"],
                          "ln2", ts_pool=False)
                for bb in range(EC):
                    tp = ppt.tile([P, P], F32, tag="tp")
                    nc.tensor.transpose(
                        tp[:], xN_new[:, tcc, bb * P : (bb + 1) * P], ident_t[:]
                    )
                    evac(xT_new[:, bb, tcc * P : (tcc + 1) * P], tp[:], bb % 4 != 0)

            ffn_pre(0)
            ffn_pre(1)
            pending()          # head 7, batch 1 attention tail
            pending = None
            ffn_mid(0)
            ffn_pre(2)
            ffn_pre(3)
            ffn_mid(1)
            for tcc in range(TC):
                ffn_post(tcc)
            xN = xN_new
            xT = xT_new
        else:
            if pending is not None:
                pending()
                pending = None
            # ---- last layer FFN on the 2 final tokens only ----
            ao_l = out_p.tile([BPC, E], F32)
            for b in range(BPC):
                nc.sync.dma_start(out=ao_l[b : b + 1, :], in_=ao_b[b][:])
            x_l = out_p.tile([BPC, E], F32)
            for b in range(BPC):
                nc.sync.dma_start(
                    out=x_l[b : b + 1, :], in_=xN[P - 1 : P, 2 * b + 1, :]
                )
            s1 = out_p.tile([BPC, E], F32)
            nc.vector.tensor_add(out=s1[:], in0=ao_l[:], in1=x_l[:])
            h1_l = out_p.tile([BPC, E], F32)
            layernorm(s1[:], h1_l[:], None, None, "lnL1", rows=BPC)
            h1T_l = ff_p.tile([P, EC, BPC], MM_DT, tag="h1Tl")
            for bb in range(EC):
                tp = ppt.tile([P, BPC], F32, tag="tp")
                nc.tensor.transpose(
                    tp[:], h1_l[:, bb * P : (bb + 1) * P], ident_t[0:BPC, 0:BPC]
                )
                evac(h1T_l[:, bb, :], tp[:], bb % 2)
            r1T_l = ff_p.tile([P, EC, BPC], MM_DT, tag="r1Tl")
            for fc in range(EC):
                ps = ppt.tile([P, BPC], F32, tag="tp")
                for ec in range(EC):
                    _mm(nc, ps[:], ff1w_t[:, ec, fc * P : (fc + 1) * P],
                        h1T_l[:, ec, :], ec == 0, ec == EC - 1)
                nc.scalar.activation(
                    out=_c(r1T_l[:, fc, :]), in_=ps[:], func=ACTF.Relu,
                    bias=biases["ff1b"][:, fc : fc + 1],
                )
            ps2 = ppb.tile([BPC, E], F32, tag="ppb")
            for fc in range(EC):
                _mm(nc, ps2[:], r1T_l[:, fc, :], ff2w_t[:, fc, :],
                    fc == 0, fc == EC - 1)
            s2 = out_p.tile([BPC, E], F32)
            nc.vector.tensor_add(out=s2[:], in0=ps2[:], in1=biases["ff2b"][:BPC, :])
            nc.vector.tensor_add(out=s2[:], in0=s2[:], in1=ao_l[:])
            xl = out_p.tile([BPC, E], F32)
            layernorm(s2[:], xl[:], biases["ln2w"], biases["ln2b"], "lnL2",
                      rows=BPC)
            xlT = ff_p.tile([P, EC, BPC], MM_DT, tag="xlT")
            for bb in range(EC):
                tp = ppt.tile([P, BPC], F32, tag="tp")
                nc.tensor.transpose(
                    tp[:], xl[:, bb * P : (bb + 1) * P], ident_t[0:BPC, 0:BPC]
                )
                evac(xlT[:, bb, :], tp[:], bb % 2)

    # ---- output head: last token of each batch ----
    wout_t = wff_p.tile([P, EC, V], MM_DT, tag="wout")
    nc.sync.dma_start(
        out=_c(wout_t[:]), in_=_c(wout.rearrange("(c p) o -> p c o", p=P))
    )
    bout_t = out_p.tile([BPC, V], F32)
    nc.sync.dma_start(out=bout_t[:], in_=bout.partition_broadcast(BPC))
    pl = ppb.tile([BPC, V], F32, tag="ppb")
    if nlayers == L and last_opt:
        xl_lhs = xlT
        cols = slice(0, BPC)
    else:
        xl_lhs = xT
        cols = slice(T - 1, TOK, T)
    for ec in range(EC):
        _mm(nc, pl[:], xl_lhs[:, ec, cols], wout_t[:, ec, :], ec == 0, ec == EC - 1)
    logits = out_p.tile([BPC, V], F32)
    nc.vector.tensor_add(out=logits[:], in0=pl[:], in1=bout_t[:])
    nmax = out_p.tile([BPC, 1], F32)
    nc.vector.reduce_max(out=nmax[:], in_=logits[:], axis=AX.X, negate=True)
    den = out_p.tile([BPC, 1], F32)
    nc.scalar.activation(
        out=logits[:], in_=logits[:], func=ACTF.Exp,
        bias=nmax[:, 0:1], accum_out=den[:],
    )
    rec = out_p.tile([BPC, 1], F32)
    nc.vector.reciprocal(out=rec[:], in_=den[:])
    nc.vector.tensor_scalar_mul(out=logits[:], in0=logits[:], scalar1=rec[:])
    nc.sync.dma_start(out=probs, in_=logits[:])


def _pe_table():
    i = np.arange(E, dtype=np.float32)
    rates = (1.0 / np.power(np.float32(10000.0), 2.0 * np.floor(i / 2.0) / E)).astype(
        np.float32
    )
    ang = np.arange(T, dtype=np.float32)[:, None] * rates[None, :]
    pe = np.concatenate([np.sin(ang[:, 0::2]), np.cos(ang[:, 1::2])], axis=-1)
    return np.tile(pe.astype(np.float32), (BPC, 1))  # [TOK, E]


def _maskd():
    j = np.arange(P)
    return np.where(j[None, :] < j[:, None], np.float32(NEG), np.float32(0.0))


def _prep_in_maps(
    input_tokens, emb, wq, wk, wv, wo, bo, ln1_w, ln1_b, ln2_w, ln2_b,
    ff1_w, ff1_b, ff2_w, ff2_b, wout, bout,
):
    f = lambda x: np.ascontiguousarray(np.asarray(x, dtype=np.float32))
    w = lambda x: np.ascontiguousarray(np.asarray(x, dtype=np.float32).astype(NP_WDT))
    wq_ = np.asarray(wq, dtype=np.float32)
    wk_ = np.asarray(wk, dtype=np.float32)
    wv_ = np.asarray(wv, dtype=np.float32)
    wo_ = np.asarray(wo, dtype=np.float32).reshape(L, H, E, E)
    wqk = np.matmul(wq_, np.swapaxes(wk_, -1, -2))
    wvo = np.matmul(wv_, wo_)
    # fold LN1's affine params into ff1 (exact):
    # relu((x*w1+b1) @ W + b) == relu(x @ (w1[:,None]*W) + (b1 @ W + b))
    ln1_w_ = np.asarray(ln1_w, dtype=np.float32)
    ln1_b_ = np.asarray(ln1_b, dtype=np.float32)
    ff1w_ = np.asarray(ff1_w, dtype=np.float32) * ln1_w_[:, :, None]
    ff1b_ = np.asarray(ff1_b, dtype=np.float32) + np.einsum(
        "le,leo->lo", ln1_b_, np.asarray(ff1_w, dtype=np.float32)
    )
    # bo is pre-added into xN on device (s1 path) and into ff2b here (s2 path)
    ff2b_ = np.asarray(ff2_b, dtype=np.float32) + np.asarray(bo, dtype=np.float32)
    toks = np.asarray(input_tokens).astype(np.int64)
    shared = {
        "emb": f(emb), "wqk": w(wqk), "wvo": w(wvo),
        "bo": f(bo), "ln2w": f(ln2_w),
        "ln2b": f(ln2_b), "ff1w": w(ff1w_), "ff1b": f(ff1b_), "ff2w": w(ff2_w),
        "ff2b": f(ff2b_), "wout": w(wout), "bout": f(bout),
        "pe2": _pe_table(), "maskd": _maskd(),
        "ident": np.eye(P, dtype=np.float32),
    }
    in_maps = []
    for c in range(NCORES):
        t = toks[c * BPC : (c + 1) * BPC].reshape(TOK)  # [512] flat tokens
        tokarr = np.ascontiguousarray(t.reshape(TC, P).T.astype(np.int32))
        in_maps.append({**shared, "tok": tokarr})
    return in_maps


def kernel(**inputs):
    if "nc" not in _CACHE:
        _CACHE["nc"] = _build()
    nc = _CACHE["nc"]
    in_maps = _prep_in_maps(**inputs)
    res = run_bass_kernel_spmd(nc, in_maps, core_ids=list(range(NCORES)))
    _CACHE["last_results"] = res
    out = np.concatenate([res.results[c]["probs"] for c in range(NCORES)], axis=0)
    return out.astype(np.float32)
